# revision 1
# baseline (speedup 1.0000x reference)
"""Trainium2 Bass kernel for nn_ExampleLabelWeights (segment_reduce).

Computes: gather per-example weight rows + cardinality, masked softmax over
each row's valid slots, weighted sum of losses, global scalar sum.

Strategy (8 NeuronCores, data-parallel over the batch):
  - batch rows (131072) are split 16384/core.
  - the params table is packed host-side into 20-float rows
    [w0..w15, cardinality_f32, pad, pad, pad] (80B, 16B aligned) and
    replicated to every core, so ONE indirect-DMA descriptor per batch row
    fetches both the weights and the cardinality.
  - on-device per core: indirect gather (GPSIMD SWDGE) -> exp on ACT ->
    mask build + masked reduces + reciprocal on DVE -> per-row ratio ->
    per-core scalar via PE matmul with ones.
  - host sums the 8 per-core partials (the only cross-core reduction).

Written in raw bass (explicit engine programs + semaphores): the walrus
build in this container only supports ONE sync-wait command per
instruction, which TileContext's auto-generated semaphores violate.
"""

from contextlib import ExitStack

import numpy as np

import concourse.bass as bass
import concourse.mybir as mybir
from concourse.bass_utils import run_bass_kernel_spmd

F32 = mybir.dt.float32
I32 = mybir.dt.int32

NCORES = 8
B = 131072
MAXC = 16
V = 1_000_000
W = 20                 # packed table row width (16 weights + card + 3 pad)
P = 128                # SBUF partitions
BC = B // NCORES       # rows per core
COLS = BC // P         # row-groups per partition (128)
CHUNKS = 2
CC = COLS // CHUNKS    # row-groups per chunk per partition


def build_kernel(chunks: int = CHUNKS, debug: bool = False):
    cc = COLS // chunks
    nc = bass.Bass()
    ptab = nc.declare_dram_parameter("ptab", [V, W], F32, isOutput=False)
    idx = nc.declare_dram_parameter("idx", [P, COLS], I32, isOutput=False)
    losses = nc.declare_dram_parameter("losses", [P, COLS * MAXC], F32,
                                       isOutput=False)
    out = nc.declare_dram_parameter("out", [1, 1], F32, isOutput=True)
    dbg = {}
    if debug:
        for name, wdt in [("iota", MAXC), ("card", 1), ("ek", MAXC),
                          ("mk", MAXC), ("den", 1), ("num", 1), ("pk", W)]:
            dbg[name] = nc.declare_dram_parameter(
                f"dbg_{name}", [P, cc * wdt], F32, isOutput=True)

    with ExitStack() as ctx:
        sem_idx = ctx.enter_context(nc.semaphore("sem_idx"))
        sem_iota = ctx.enter_context(nc.semaphore("sem_iota"))
        sem_g = [ctx.enter_context(nc.semaphore(f"sem_g{k}"))
                 for k in range(chunks)]
        sem_l = [ctx.enter_context(nc.semaphore(f"sem_l{k}"))
                 for k in range(chunks)]
        sem_exp = ctx.enter_context(nc.semaphore("sem_exp"))
        sem_dve = ctx.enter_context(nc.semaphore("sem_dve"))
        sem_mm = ctx.enter_context(nc.semaphore("sem_mm"))
        sem_res = ctx.enter_context(nc.semaphore("sem_res"))
        sem_out = ctx.enter_context(nc.semaphore("sem_out"))
        all_sems = [sem_idx, sem_iota, *sem_g, *sem_l, sem_exp,
                    sem_dve, sem_mm, sem_res, sem_out]

        idxt = ctx.enter_context(nc.sbuf_tensor("idxt", [P, COLS], I32))
        ioti = ctx.enter_context(nc.sbuf_tensor("ioti", [P, cc * MAXC], I32))
        iotat = ctx.enter_context(nc.sbuf_tensor("iotat", [P, cc * MAXC], F32))
        ones = ctx.enter_context(nc.sbuf_tensor("ones", [P, 1], F32))
        acc = ctx.enter_context(nc.sbuf_tensor("acc", [P, COLS], F32))
        colsum = ctx.enter_context(nc.sbuf_tensor("colsum", [P, 1], F32))
        res = ctx.enter_context(nc.sbuf_tensor("res", [1, 1], F32))
        tot = ctx.enter_context(nc.psum_tensor("tot", [1, 1], F32))

        pk, lk, ek, mk, em, nm, cardt, den, num, rd = ([] for _ in range(10))
        for k in range(chunks):
            pk.append(ctx.enter_context(
                nc.sbuf_tensor(f"pk{k}", [P, cc * W], F32)))
            lk.append(ctx.enter_context(
                nc.sbuf_tensor(f"lk{k}", [P, cc * MAXC], F32)))
            ek.append(ctx.enter_context(
                nc.sbuf_tensor(f"ek{k}", [P, cc * MAXC], F32)))
            mk.append(ctx.enter_context(
                nc.sbuf_tensor(f"mk{k}", [P, cc * MAXC], F32)))
            em.append(ctx.enter_context(
                nc.sbuf_tensor(f"em{k}", [P, cc * MAXC], F32)))
            nm.append(ctx.enter_context(
                nc.sbuf_tensor(f"nm{k}", [P, cc * MAXC], F32)))
            cardt.append(ctx.enter_context(
                nc.sbuf_tensor(f"cardt{k}", [P, cc], F32)))
            den.append(ctx.enter_context(
                nc.sbuf_tensor(f"den{k}", [P, cc], F32)))
            num.append(ctx.enter_context(
                nc.sbuf_tensor(f"num{k}", [P, cc], F32)))
            rd.append(ctx.enter_context(
                nc.sbuf_tensor(f"rd{k}", [P, cc], F32)))

        def r3(ap, width):
            return ap.rearrange("p (c u) -> p c u", u=width)

        with nc.Block(no_gpsimd_drain=True) as block:

            @block.sync
            def _(sync):
                sync.dma_start(out=idxt[:, :], in_=idx[:, :]).then_inc(
                    sem_idx, 16)
                for k in range(chunks):
                    sync.dma_start(
                        out=lk[k][:, :],
                        in_=losses[:, k * cc * MAXC:(k + 1) * cc * MAXC],
                    ).then_inc(sem_l[k], 16)
                sync.wait_ge(sem_res, 1)
                sync.dma_start(out=out[:, :], in_=res[:, :]).then_inc(
                    sem_out, 16)
                n_out = 16
                if debug:
                    for name, src in [("iota", iotat), ("card", cardt[0]),
                                      ("ek", ek[0]), ("mk", mk[0]),
                                      ("den", den[0]), ("num", num[0]),
                                      ("pk", pk[0])]:
                        sync.dma_start(
                            out=dbg[name][:, :], in_=src[:, :]
                        ).then_inc(sem_out, 16)
                        n_out += 16
                sync.wait_ge(sem_out, n_out)

            @block.gpsimd
            def _(gpsimd):
                gpsimd.iota(
                    ioti[:, :], pattern=[[0, cc], [1, MAXC]],
                    base=0, channel_multiplier=0,
                ).then_inc(sem_iota, 1)
                gpsimd.wait_ge(sem_idx, 16)
                for k in range(chunks):
                    gpsimd.indirect_dma_start(
                        out=pk[k][:, :],
                        out_offset=None,
                        in_=ptab[:, :],
                        in_offset=bass.IndirectOffsetOnAxis(
                            ap=idxt[:, k * cc:(k + 1) * cc], axis=0
                        ),
                    ).then_inc(sem_g[k], 16)

            @block.scalar
            def _(scalar):
                for k in range(chunks):
                    scalar.wait_ge(sem_g[k], 16)
                    scalar.activation(
                        out=r3(ek[k][:, :], MAXC)[:, :, :],
                        in_=r3(pk[k][:, :], W)[:, :, 0:MAXC],
                        func=mybir.ActivationFunctionType.Exp,
                    ).then_inc(sem_exp, 1)

            # The DVE pipeline does not interlock same-engine RAW hazards:
            # every dependent pair needs an explicit wait on the engine's
            # completion counter. Track producer indices at build time and
            # emit monotone wait_ge's (skipping already-covered thresholds).
            marks = {}

            @block.vector
            def _(vector):
                state = {"n": 0, "hw": 0}

                def bump(inst):
                    state["n"] += 1
                    inst.then_inc(sem_dve, 1)
                    return state["n"]

                def dep(*ths):
                    th = max(ths)
                    if th > state["hw"]:
                        vector.wait_ge(sem_dve, th)
                        state["hw"] = th

                vector.wait_ge(sem_iota, 1)
                i_iotat = bump(vector.tensor_copy(out=iotat[:, :],
                                                  in_=ioti[:, :]))
                bump(vector.memset(ones[:, :], 1.0))
                for k in range(chunks):
                    vector.wait_ge(sem_g[k], 16)
                    i_card = bump(vector.tensor_copy(
                        out=r3(cardt[k][:, :], 1)[:, :, :],
                        in_=r3(pk[k][:, :], W)[:, :, MAXC:MAXC + 1],
                    ))
                    dep(i_card, i_iotat)
                    i_mk = bump(vector.tensor_tensor(
                        out=r3(mk[k][:, :], MAXC)[:, :, :],
                        in0=r3(cardt[k][:, :], 1).broadcast_to([P, cc, MAXC]),
                        in1=r3(iotat[:, :], MAXC)[:, :, :],
                        op=mybir.AluOpType.is_gt,
                    ))
                    vector.wait_ge(sem_exp, k + 1)
                    dep(i_mk)
                    i_em = bump(vector.tensor_tensor(
                        out=em[k][:, :], in0=ek[k][:, :], in1=mk[k][:, :],
                        op=mybir.AluOpType.mult,
                    ))
                    dep(i_em)
                    i_den = bump(vector.tensor_reduce(
                        out=den[k][:, :], in_=r3(em[k][:, :], MAXC)[:, :, :],
                        axis=mybir.AxisListType.X, op=mybir.AluOpType.add,
                    ))
                    vector.wait_ge(sem_l[k], 16)
                    i_nm = bump(vector.tensor_tensor(
                        out=nm[k][:, :], in0=em[k][:, :], in1=lk[k][:, :],
                        op=mybir.AluOpType.mult,
                    ))
                    dep(i_nm)
                    i_num = bump(vector.tensor_reduce(
                        out=num[k][:, :], in_=r3(nm[k][:, :], MAXC)[:, :, :],
                        axis=mybir.AxisListType.X, op=mybir.AluOpType.add,
                    ))
                    dep(i_den)
                    i_rd = bump(vector.reciprocal(out=rd[k][:, :],
                                                  in_=den[k][:, :]))
                    dep(i_num, i_rd)
                    bump(vector.tensor_tensor(
                        out=acc[:, k * cc:(k + 1) * cc],
                        in0=num[k][:, :], in1=rd[k][:, :],
                        op=mybir.AluOpType.mult,
                    ))
                dep(state["n"])
                i_colsum = bump(vector.tensor_reduce(
                    out=colsum[:, :], in_=acc[:, :],
                    axis=mybir.AxisListType.X, op=mybir.AluOpType.add,
                ))
                marks["colsum"] = i_colsum
                vector.wait_ge(sem_mm, 1)
                vector.tensor_copy(out=res[:, :], in_=tot[:, :]).then_inc(
                    sem_res, 1)

            @block.tensor
            def _(tensor):
                tensor.wait_ge(sem_dve, marks["colsum"])
                tensor.matmul(
                    out=tot[:, :], lhsT=colsum[:, :], rhs=ones[:, :],
                    start=True, stop=True,
                ).then_inc(sem_mm, 1)

    return nc


def make_inputs(losses, inputs_idx, params, cardinality):
    """Pack/shard full inputs into per-core input maps."""
    ptab = np.zeros((V, W), dtype=np.float32)
    ptab[:, :MAXC] = np.asarray(params, dtype=np.float32)
    ptab[:, MAXC] = np.asarray(cardinality).astype(np.float32)
    idx_full = np.asarray(inputs_idx, dtype=np.int32)
    losses_full = np.asarray(losses, dtype=np.float32)
    in_maps = []
    for c in range(NCORES):
        sl = slice(c * BC, (c + 1) * BC)
        in_maps.append({
            "ptab": ptab,
            "idx": np.ascontiguousarray(idx_full[sl].reshape(P, COLS)),
            "losses": np.ascontiguousarray(losses_full[sl].reshape(P, COLS * MAXC)),
        })
    return in_maps


_NC_CACHE = {}


def kernel(losses, inputs_idx, params, cardinality, trace=False, **kw):
    key = CHUNKS
    if key not in _NC_CACHE:
        _NC_CACHE[key] = build_kernel(CHUNKS)
    nc = _NC_CACHE[key]
    in_maps = make_inputs(losses, inputs_idx, params, cardinality)
    r = run_bass_kernel_spmd(nc, in_maps, list(range(NCORES)), trace=trace, **kw)
    total = np.float64(0.0)
    for c in range(NCORES):
        total += np.float64(r.results[c]["out"][0, 0])
    out = np.float32(total)
    if trace:
        kernel.last_results = r
    return np.asarray(out)


kernel.last_results = None



# revision 5
# speedup vs baseline: 1.1195x; 1.1195x over previous
"""Trainium2 Bass kernel for nn_ExampleLabelWeights (segment_reduce).

Computes: gather per-example weight rows, masked softmax over each row's
valid slots, weighted sum of losses, global scalar sum.

Strategy (8 NeuronCores, data-parallel over the batch):
  - batch rows (131072) are split 16384/core.
  - the mask is folded into the params table HOST-side: invalid slots are
    set to -100 so exp() gives (effectively) 0 weight. The table is cast
    to bf16 -> 32B rows; ONE indirect-DMA descriptor per batch row.
  - losses are cast to bf16 host-side and streamed per chunk.
  - on-device per core, per chunk: indirect gather (GPSIMD SWDGE) ->
    exp on ACT -> nm=ek*lk (DVE mult) + two segmented reduces (DVE).
    Tail: one reciprocal + ratio mult + per-partition colsum (DVE).
  - per-core [128,1] partials are DMA'd out; host sums 8*128 values.

Written in raw bass (explicit engine programs + semaphores): the walrus
build in this container only supports ONE sync-wait command per
instruction, which TileContext's auto-generated semaphores violate.
"""

from contextlib import ExitStack

import numpy as np
import ml_dtypes

import concourse.bass as bass
import concourse.mybir as mybir
from concourse.bass_utils import run_bass_kernel_spmd

F32 = mybir.dt.float32
BF16 = mybir.dt.bfloat16
I32 = mybir.dt.int32

NCORES = 8
B = 131072
MAXC = 16
V = 1_000_000
P = 128                # SBUF partitions
BC = B // NCORES       # rows per core
COLS = BC // P         # row-groups per partition (128)

CHUNKS = 2
USE_BF16 = True
MASK_FILL = -100.0     # exp(-100) == 0 in f32/bf16 for all practical purposes


def build_kernel(chunks: int = CHUNKS, use_bf16: bool = USE_BF16):
    cc = COLS // chunks
    DT = BF16 if use_bf16 else F32
    nc = bass.Bass()
    ptab = nc.declare_dram_parameter("ptab", [V, MAXC], DT, isOutput=False)
    idx = nc.declare_dram_parameter("idx", [P, COLS], I32, isOutput=False)
    losses = nc.declare_dram_parameter("losses", [P, COLS * MAXC], DT,
                                       isOutput=False)
    out = nc.declare_dram_parameter("out", [P, 1], F32, isOutput=True)

    with ExitStack() as ctx:
        sem_idx = ctx.enter_context(nc.semaphore("sem_idx"))
        sem_g = [ctx.enter_context(nc.semaphore(f"sem_g{k}"))
                 for k in range(chunks)]
        sem_l = [ctx.enter_context(nc.semaphore(f"sem_l{k}"))
                 for k in range(chunks)]
        sem_exp = ctx.enter_context(nc.semaphore("sem_exp"))
        sem_dve = ctx.enter_context(nc.semaphore("sem_dve"))
        sem_res = ctx.enter_context(nc.semaphore("sem_res"))
        sem_out = ctx.enter_context(nc.semaphore("sem_out"))

        idxt = ctx.enter_context(nc.sbuf_tensor("idxt", [P, COLS], I32))
        den = ctx.enter_context(nc.sbuf_tensor("den", [P, COLS], F32))
        num = ctx.enter_context(nc.sbuf_tensor("num", [P, COLS], F32))
        rd = ctx.enter_context(nc.sbuf_tensor("rd", [P, COLS], F32))
        acc = ctx.enter_context(nc.sbuf_tensor("acc", [P, COLS], F32))
        colsum = ctx.enter_context(nc.sbuf_tensor("colsum", [P, 1], F32))
        scratch = ctx.enter_context(nc.sbuf_tensor("scratch", [P, 1], F32))

        pk, lk, ek, nm = ([] for _ in range(4))
        for k in range(chunks):
            pk.append(ctx.enter_context(
                nc.sbuf_tensor(f"pk{k}", [P, cc * MAXC], DT)))
            lk.append(ctx.enter_context(
                nc.sbuf_tensor(f"lk{k}", [P, cc * MAXC], DT)))
            ek.append(ctx.enter_context(
                nc.sbuf_tensor(f"ek{k}", [P, cc * MAXC], DT)))
            nm.append(ctx.enter_context(
                nc.sbuf_tensor(f"nm{k}", [P, cc * MAXC], DT)))

        def r3(ap, width):
            return ap.rearrange("p (c u) -> p c u", u=width)

        marks = {}

        with nc.Block(no_gpsimd_drain=True) as block:

            @block.sync
            def _(sync):
                sync.dma_start(out=idxt[:, :], in_=idx[:, :]).then_inc(
                    sem_idx, 16)
                for k in range(chunks):
                    sync.dma_start(
                        out=lk[k][:, :],
                        in_=losses[:, k * cc * MAXC:(k + 1) * cc * MAXC],
                    ).then_inc(sem_l[k], 16)
                sync.wait_ge(sem_res, 1)
                sync.dma_start(out=out[:, :], in_=colsum[:, :]).then_inc(
                    sem_out, 16)
                sync.wait_ge(sem_out, 16)

            @block.gpsimd
            def _(gpsimd):
                gpsimd.wait_ge(sem_idx, 16)
                for k in range(chunks):
                    gpsimd.indirect_dma_start(
                        out=pk[k][:, :],
                        out_offset=None,
                        in_=ptab[:, :],
                        in_offset=bass.IndirectOffsetOnAxis(
                            ap=idxt[:, k * cc:(k + 1) * cc], axis=0
                        ),
                    ).then_inc(sem_g[k], 16)

            @block.scalar
            def _(scalar):
                # dummy activation: forces the EXP table load off the
                # critical path (overlaps the idx DMA + gather).
                scalar.activation(
                    out=scratch[:, :], in_=scratch[:, :],
                    func=mybir.ActivationFunctionType.Exp,
                )
                for k in range(chunks):
                    scalar.wait_ge(sem_g[k], 16)
                    scalar.activation(
                        out=ek[k][:, :], in_=pk[k][:, :],
                        func=mybir.ActivationFunctionType.Exp,
                    ).then_inc(sem_exp, 1)

            # The DVE pipeline does not interlock same-engine RAW hazards:
            # every dependent pair needs an explicit wait on the engine's
            # completion counter.
            @block.vector
            def _(vector):
                state = {"n": 0, "hw": 0}

                def bump(inst):
                    state["n"] += 1
                    inst.then_inc(sem_dve, 1)
                    return state["n"]

                def dep(*ths):
                    th = max(ths)
                    if th > state["hw"]:
                        vector.wait_ge(sem_dve, th)
                        state["hw"] = th

                i_mult = [0] * chunks
                i_den = [0] * chunks
                i_num = [0] * chunks
                for k in range(chunks):
                    sl = slice(k * cc, (k + 1) * cc)
                    vector.wait_ge(sem_exp, k + 1)
                    i_den[k] = bump(vector.tensor_reduce(
                        out=den[:, sl], in_=r3(ek[k][:, :], MAXC)[:, :, :],
                        axis=mybir.AxisListType.X, op=mybir.AluOpType.add,
                    ))
                    vector.wait_ge(sem_l[k], 16)
                    i_mult[k] = bump(vector.tensor_tensor(
                        out=nm[k][:, :], in0=ek[k][:, :], in1=lk[k][:, :],
                        op=mybir.AluOpType.mult,
                    ))
                    dep(i_mult[k])
                    i_num[k] = bump(vector.tensor_reduce(
                        out=num[:, sl], in_=r3(nm[k][:, :], MAXC)[:, :, :],
                        axis=mybir.AxisListType.X, op=mybir.AluOpType.add,
                    ))
                dep(*i_den)
                i_rd = bump(vector.reciprocal(out=rd[:, :], in_=den[:, :]))
                dep(i_rd, *i_num)
                i_acc = bump(vector.tensor_tensor(
                    out=acc[:, :], in0=num[:, :], in1=rd[:, :],
                    op=mybir.AluOpType.mult,
                ))
                dep(i_acc)
                vector.tensor_reduce(
                    out=colsum[:, :], in_=acc[:, :],
                    axis=mybir.AxisListType.X, op=mybir.AluOpType.add,
                ).then_inc(sem_res, 1)

    return nc


def make_inputs(losses, inputs_idx, params, cardinality,
                use_bf16: bool = USE_BF16):
    """Pack/shard full inputs into per-core input maps."""
    npdt = ml_dtypes.bfloat16 if use_bf16 else np.float32
    p = np.asarray(params, dtype=np.float32)
    card = np.asarray(cardinality, dtype=np.int32)
    mask = np.arange(MAXC, dtype=np.int32)[None, :] < card[:, None]
    ptab = np.where(mask, p, np.float32(MASK_FILL)).astype(npdt)
    idx_full = np.asarray(inputs_idx, dtype=np.int32)
    losses_full = np.asarray(losses, dtype=np.float32).astype(npdt)
    in_maps = []
    for c in range(NCORES):
        sl = slice(c * BC, (c + 1) * BC)
        in_maps.append({
            "ptab": ptab,
            "idx": np.ascontiguousarray(idx_full[sl].reshape(P, COLS)),
            "losses": np.ascontiguousarray(
                losses_full[sl].reshape(P, COLS * MAXC)),
        })
    return in_maps


_NC_CACHE = {}


def kernel(losses, inputs_idx, params, cardinality, trace=False, **kw):
    key = (CHUNKS, USE_BF16)
    if key not in _NC_CACHE:
        _NC_CACHE[key] = build_kernel(CHUNKS, USE_BF16)
    nc = _NC_CACHE[key]
    in_maps = make_inputs(losses, inputs_idx, params, cardinality, USE_BF16)
    r = run_bass_kernel_spmd(nc, in_maps, list(range(NCORES)), trace=trace, **kw)
    total = np.float64(0.0)
    for c in range(NCORES):
        total += np.float64(np.asarray(r.results[c]["out"],
                                       dtype=np.float32).sum(dtype=np.float64))
    out = np.float32(total)
    if trace:
        kernel.last_results = r
    return np.asarray(out)


kernel.last_results = None


# revision 7
# speedup vs baseline: 1.3257x; 1.1842x over previous
"""Trainium2 Bass kernel for nn_ExampleLabelWeights (segment_reduce).

Computes: gather per-example weight rows, masked softmax over each row's
valid slots, weighted sum of losses, global scalar sum.

Strategy (8 NeuronCores, data-parallel over the batch):
  - batch rows (131072) are split 16384/core; per core the (idx, losses)
    pairs are SORTED by idx host-side (the final sum is permutation
    invariant) so the indirect gather walks monotone addresses.
  - the mask is folded into the params table HOST-side: invalid slots are
    set to -100 so exp() gives (effectively) 0 weight. The table is cast
    to bf16 -> 32B rows; ONE indirect-DMA descriptor per batch row.
  - losses are cast to bf16 and packed chunk-major host-side.
  - on-device per core, per chunk: indirect gather (GPSIMD SWDGE) ->
    exp on ACT into the lo half of a combo buffer -> nm=ek*lk (DVE mult,
    2x bf16) into the hi half -> ONE fused segmented reduce produces
    den|num. Tail: reciprocal + ratio + colsum (DVE), partition-sum via
    PE matmul with ones, single 4B result DMA out.
  - host sums the 8 per-core scalars.

Written in raw bass (explicit engine programs + semaphores): the walrus
build in this container only supports ONE sync-wait command per
instruction, which TileContext's auto-generated semaphores violate.
"""

from contextlib import ExitStack

import numpy as np
import ml_dtypes

import concourse.bass as bass
import concourse.mybir as mybir
from concourse.bass_utils import run_bass_kernel_spmd

F32 = mybir.dt.float32
BF16 = mybir.dt.bfloat16
I32 = mybir.dt.int32

NCORES = 8
B = 131072
MAXC = 16
V = 1_000_000
P = 128                # SBUF partitions
BC = B // NCORES       # rows per core
COLS = BC // P         # row-groups per partition (128)

CHUNKS = 4
USE_BF16 = True
SORT = True
MASK_FILL = -100.0     # exp(-100) == 0 in f32/bf16 for all practical purposes


def build_kernel(chunks: int = CHUNKS, use_bf16: bool = USE_BF16):
    cc = COLS // chunks
    n = cc * MAXC          # elems per chunk per partition
    DT = BF16 if use_bf16 else F32
    nc = bass.Bass()
    ptab = nc.declare_dram_parameter("ptab", [V, MAXC], DT, isOutput=False)
    # idx is packed chunk-major host-side: row block k*P..(k+1)*P-1 holds
    # chunk-pair k's [P, 2*cc] indices (two chunks per DMA).
    idx = nc.declare_dram_parameter("idx", [2 * P, 2 * cc], I32,
                                    isOutput=False)
    losses = nc.declare_dram_parameter("losses", [chunks * P, n], DT,
                                       isOutput=False)
    out = nc.declare_dram_parameter("out", [1, 1], F32, isOutput=True)

    with ExitStack() as ctx:
        sem_idx = [ctx.enter_context(nc.semaphore(f"sem_idx{h}"))
                   for h in range(2)]
        sem_g = [ctx.enter_context(nc.semaphore(f"sem_g{k}"))
                 for k in range(chunks)]
        sem_l = [ctx.enter_context(nc.semaphore(f"sem_l{k}"))
                 for k in range(chunks)]
        sem_w = ctx.enter_context(nc.semaphore("sem_w"))
        sem_exp = ctx.enter_context(nc.semaphore("sem_exp"))
        sem_dve = ctx.enter_context(nc.semaphore("sem_dve"))
        sem_res = ctx.enter_context(nc.semaphore("sem_res"))
        sem_mm = ctx.enter_context(nc.semaphore("sem_mm"))
        sem_res2 = ctx.enter_context(nc.semaphore("sem_res2"))
        sem_out = ctx.enter_context(nc.semaphore("sem_out"))

        idxt = ctx.enter_context(nc.sbuf_tensor("idxt", [P, COLS], I32))
        widx = ctx.enter_context(nc.sbuf_tensor("widx", [P, 2], I32))
        wbuf = ctx.enter_context(nc.sbuf_tensor("wbuf", [P, 2 * MAXC], DT))
        # dennum[:, k*2cc : k*2cc+cc] = den chunk k; +cc = num chunk k
        dennum = ctx.enter_context(
            nc.sbuf_tensor("dennum", [P, 2 * COLS], F32))
        rd = ctx.enter_context(nc.sbuf_tensor("rd", [P, COLS], F32))
        acc = ctx.enter_context(nc.sbuf_tensor("acc", [P, COLS], F32))
        colsum = ctx.enter_context(nc.sbuf_tensor("colsum", [P, 1], F32))
        ones = ctx.enter_context(nc.sbuf_tensor("ones", [P, 1], F32))
        res = ctx.enter_context(nc.sbuf_tensor("res", [1, 1], F32))
        scratch = ctx.enter_context(nc.sbuf_tensor("scratch", [P, 1], F32))
        tot = ctx.enter_context(nc.psum_tensor("tot", [1, 1], F32))

        pk, lk, combo = ([] for _ in range(3))
        for k in range(chunks):
            pk.append(ctx.enter_context(
                nc.sbuf_tensor(f"pk{k}", [P, n], DT)))
            lk.append(ctx.enter_context(
                nc.sbuf_tensor(f"lk{k}", [P, n], DT)))
            # combo: [ek | nm] so one reduce covers both
            combo.append(ctx.enter_context(
                nc.sbuf_tensor(f"combo{k}", [P, 2 * n], DT)))

        def r3(ap, width):
            return ap.rearrange("p (c u) -> p c u", u=width)

        with nc.Block(no_gpsimd_drain=True) as block:

            @block.sync
            def _(sync):
                for h in range(2):
                    sync.dma_start(
                        out=idxt[:, h * 2 * cc:(h + 1) * 2 * cc],
                        in_=idx[h * P:(h + 1) * P, :],
                    ).then_inc(sem_idx[h], 16)
                for k in range(2):
                    sync.dma_start(
                        out=lk[k][:, :],
                        in_=losses[k * P:(k + 1) * P, :],
                    ).then_inc(sem_l[k], 16)
                sync.wait_ge(sem_res2, 1)
                sync.dma_start(out=out[:, :], in_=res[:, :]).then_inc(
                    sem_out, 16)
                sync.wait_ge(sem_out, 16)

            @block.gpsimd
            def _(gpsimd):
                # warm up the SWDGE path (Q7 library load + queue start)
                # with a tiny gather of row 0 before the real indices land.
                gpsimd.memset(widx[:, :], 0).then_inc(sem_w, 1)
                gpsimd.wait_ge(sem_w, 1)
                gpsimd.indirect_dma_start(
                    out=wbuf[:, :],
                    out_offset=None,
                    in_=ptab[:, :],
                    in_offset=bass.IndirectOffsetOnAxis(
                        ap=widx[:, :], axis=0),
                ).then_inc(sem_w, 16)
                for k in range(chunks):
                    gpsimd.wait_ge(sem_idx[k // 2], 16)
                    gpsimd.indirect_dma_start(
                        out=pk[k][:, :],
                        out_offset=None,
                        in_=ptab[:, :],
                        in_offset=bass.IndirectOffsetOnAxis(
                            ap=idxt[:, k * cc:(k + 1) * cc], axis=0
                        ),
                    ).then_inc(sem_g[k], 16)

            @block.scalar
            def _(scalar):
                # two loss chunks stream from the scalar engine's queue
                for k in range(2, chunks):
                    scalar.dma_start(
                        out=lk[k][:, :],
                        in_=losses[k * P:(k + 1) * P, :],
                    ).then_inc(sem_l[k], 16)
                # dummy activation: forces the EXP table load off the
                # critical path (overlaps the idx DMA + gather).
                scalar.activation(
                    out=scratch[:, :], in_=scratch[:, :],
                    func=mybir.ActivationFunctionType.Exp,
                )
                for k in range(chunks):
                    scalar.wait_ge(sem_g[k], 16)
                    scalar.activation(
                        out=combo[k][:, 0:n], in_=pk[k][:, :],
                        func=mybir.ActivationFunctionType.Exp,
                    ).then_inc(sem_exp, 1)

            # The DVE pipeline does not interlock same-engine RAW hazards:
            # every dependent pair needs an explicit wait on the engine's
            # completion counter.
            @block.vector
            def _(vector):
                state = {"n": 0, "hw": 0}

                def bump(inst):
                    state["n"] += 1
                    inst.then_inc(sem_dve, 1)
                    return state["n"]

                def dep(*ths):
                    th = max(ths)
                    if th > state["hw"]:
                        vector.wait_ge(sem_dve, th)
                        state["hw"] = th

                bump(vector.memset(ones[:, :], 1.0))
                i_red = [0] * chunks
                for k in range(chunks):
                    vector.wait_ge(sem_exp, k + 1)
                    vector.wait_ge(sem_l[k], 16)
                    i_mult = bump(vector.tensor_tensor(
                        out=combo[k][:, n:2 * n],
                        in0=combo[k][:, 0:n], in1=lk[k][:, :],
                        op=mybir.AluOpType.mult,
                    ))
                    dep(i_mult)
                    i_red[k] = bump(vector.tensor_reduce(
                        out=dennum[:, k * 2 * cc:(k + 1) * 2 * cc],
                        in_=r3(combo[k][:, :], MAXC)[:, :, :],
                        axis=mybir.AxisListType.X, op=mybir.AluOpType.add,
                    ))
                den_v = r3(dennum[:, :], 2 * cc)[:, :, 0:cc]
                num_v = r3(dennum[:, :], 2 * cc)[:, :, cc:2 * cc]
                rd_v = r3(rd[:, :], cc)
                acc_v = r3(acc[:, :], cc)
                dep(*i_red)
                i_rd = bump(vector.reciprocal(out=rd_v[:, :, :],
                                              in_=den_v))
                dep(i_rd)
                i_acc = bump(vector.tensor_tensor(
                    out=acc_v[:, :, :], in0=num_v, in1=rd_v[:, :, :],
                    op=mybir.AluOpType.mult,
                ))
                dep(i_acc)
                vector.tensor_reduce(
                    out=colsum[:, :], in_=acc[:, :],
                    axis=mybir.AxisListType.X, op=mybir.AluOpType.add,
                ).then_inc(sem_res, 1)
                vector.wait_ge(sem_mm, 1)
                vector.tensor_copy(out=res[:, :], in_=tot[:, :]).then_inc(
                    sem_res2, 1)

            @block.tensor
            def _(tensor):
                tensor.wait_ge(sem_res, 1)
                tensor.matmul(
                    out=tot[:, :], lhsT=colsum[:, :], rhs=ones[:, :],
                    start=True, stop=True,
                ).then_inc(sem_mm, 1)

    return nc


def make_inputs(losses, inputs_idx, params, cardinality,
                chunks: int = CHUNKS, use_bf16: bool = USE_BF16,
                sort: bool = SORT):
    """Pack/shard full inputs into per-core input maps."""
    npdt = ml_dtypes.bfloat16 if use_bf16 else np.float32
    cc = COLS // chunks
    n = cc * MAXC
    p = np.asarray(params, dtype=np.float32)
    card = np.asarray(cardinality, dtype=np.int32)
    mask = np.arange(MAXC, dtype=np.int32)[None, :] < card[:, None]
    ptab = np.where(mask, p, np.float32(MASK_FILL)).astype(npdt)
    idx_full = np.asarray(inputs_idx, dtype=np.int32)
    losses_full = np.asarray(losses, dtype=np.float32)
    in_maps = []
    for c in range(NCORES):
        sl = slice(c * BC, (c + 1) * BC)
        idx_c = idx_full[sl]
        losses_c = losses_full[sl]
        if sort:
            order = np.argsort(idx_c)
            idx_c = idx_c[order]
            losses_c = losses_c[order]
        # chunk-major packing: chunk k holds sorted ranks
        # [k*P*cc, (k+1)*P*cc), laid out [P, cc] row-major; on SBUF the
        # chunk sits at idxt[:, k*cc:(k+1)*cc].
        idx_r = idx_c.reshape(chunks, P, cc)
        idx_packed = np.concatenate(
            [np.concatenate([idx_r[2 * h], idx_r[2 * h + 1]], axis=1)
             for h in range(chunks // 2)], axis=0)
        losses_packed = losses_c.reshape(chunks * P, n).astype(npdt)
        in_maps.append({
            "ptab": ptab,
            "idx": np.ascontiguousarray(idx_packed),
            "losses": np.ascontiguousarray(losses_packed),
        })
    return in_maps


_NC_CACHE = {}


def kernel(losses, inputs_idx, params, cardinality, trace=False, **kw):
    key = (CHUNKS, USE_BF16)
    if key not in _NC_CACHE:
        _NC_CACHE[key] = build_kernel(CHUNKS, USE_BF16)
    nc = _NC_CACHE[key]
    in_maps = make_inputs(losses, inputs_idx, params, cardinality,
                          CHUNKS, USE_BF16, SORT)
    r = run_bass_kernel_spmd(nc, in_maps, list(range(NCORES)), trace=trace, **kw)
    total = np.float64(0.0)
    for c in range(NCORES):
        total += np.float64(np.asarray(r.results[c]["out"],
                                       dtype=np.float32).sum(dtype=np.float64))
    out = np.float32(total)
    if trace:
        kernel.last_results = r
    return np.asarray(out)


kernel.last_results = None


# revision 14
# speedup vs baseline: 1.3553x; 1.0223x over previous
"""Trainium2 Bass kernel for nn_ExampleLabelWeights (segment_reduce).

Computes: gather per-example weight rows, masked softmax over each row's
valid slots, weighted sum of losses, global scalar sum.

Strategy (8 NeuronCores, data-parallel over the batch):
  - batch rows (131072) are split 16384/core; per core the (idx, losses)
    pairs are SORTED by idx host-side (the final sum is permutation
    invariant) so the indirect gather walks monotone addresses.
  - the mask is folded into the params table HOST-side: invalid slots are
    set to -100 so exp() gives (effectively) 0 weight. The table is cast
    to bf16 -> 32B rows; ONE indirect-DMA descriptor per batch row.
  - losses are cast to bf16 and packed chunk-major host-side.
  - on-device per core, per chunk: indirect gather (GPSIMD SWDGE) ->
    exp on ACT into the lo half of a combo buffer -> nm=ek*lk (DVE mult,
    2x bf16) into the hi half -> ONE fused segmented reduce produces
    den|num. Tail: reciprocal + ratio + colsum (DVE), partition-sum via
    PE matmul with ones, single 4B result DMA out.
  - host sums the 8 per-core scalars.

Written in raw bass (explicit engine programs + semaphores): the walrus
build in this container only supports ONE sync-wait command per
instruction, which TileContext's auto-generated semaphores violate.
"""

from contextlib import ExitStack

import numpy as np
import ml_dtypes

import concourse.bass as bass
import concourse.mybir as mybir
from concourse.bass_utils import run_bass_kernel_spmd

F32 = mybir.dt.float32
BF16 = mybir.dt.bfloat16
I32 = mybir.dt.int32

NCORES = 8
B = 131072
MAXC = 16
V = 1_000_000
P = 128                # SBUF partitions
BC = B // NCORES       # rows per core
COLS = BC // P         # row-groups per partition (128)

CHUNKS = 4
USE_BF16 = True
SORT = True
USE_POOL = False       # pool needs 4B dtypes (s4d4); bf16 input rejected
USE_RECIP_APPROX = False  # custom-DVE ISA not supported by this walrus build
MASK_FILL = -100.0     # exp(-100) == 0 in f32/bf16 for all practical purposes


def build_kernel(chunks: int = CHUNKS, use_bf16: bool = USE_BF16):
    cc = COLS // chunks
    n = cc * MAXC          # elems per chunk per partition
    DT = BF16 if use_bf16 else F32
    nc = bass.Bass()
    ptab = nc.declare_dram_parameter("ptab", [V, MAXC], DT, isOutput=False)
    # idx is packed chunk-major host-side: row block k*P..(k+1)*P-1 holds
    # chunk-pair k's [P, 2*cc] indices (two chunks per DMA).
    idx = nc.declare_dram_parameter("idx", [2 * P, 2 * cc], I32,
                                    isOutput=False)
    losses = nc.declare_dram_parameter("losses", [chunks * P, n], DT,
                                       isOutput=False)
    out = nc.declare_dram_parameter("out", [1, 1], F32, isOutput=True)

    with ExitStack() as ctx:
        sem_idx = [ctx.enter_context(nc.semaphore(f"sem_idx{h}"))
                   for h in range(2)]
        sem_g = [ctx.enter_context(nc.semaphore(f"sem_g{k}"))
                 for k in range(chunks)]
        sem_l = [ctx.enter_context(nc.semaphore(f"sem_l{k}"))
                 for k in range(chunks)]
        sem_w = ctx.enter_context(nc.semaphore("sem_w"))
        sem_exp = ctx.enter_context(nc.semaphore("sem_exp"))
        sem_dve = ctx.enter_context(nc.semaphore("sem_dve"))
        sem_res = ctx.enter_context(nc.semaphore("sem_res"))
        sem_mm = ctx.enter_context(nc.semaphore("sem_mm"))
        sem_res2 = ctx.enter_context(nc.semaphore("sem_res2"))
        sem_out = ctx.enter_context(nc.semaphore("sem_out"))

        idxt = ctx.enter_context(nc.sbuf_tensor("idxt", [P, COLS], I32))
        widx = ctx.enter_context(nc.sbuf_tensor("widx", [P, 2], I32))
        wbuf = ctx.enter_context(nc.sbuf_tensor("wbuf", [P, 2 * MAXC], DT))
        # dennum[:, k*2cc : k*2cc+cc] = den chunk k; +cc = num chunk k
        dennum = ctx.enter_context(
            nc.sbuf_tensor("dennum", [P, 2 * COLS], F32))
        rd = ctx.enter_context(nc.sbuf_tensor("rd", [P, COLS], F32))
        acc = ctx.enter_context(nc.sbuf_tensor("acc", [P, COLS], F32))
        colsum = ctx.enter_context(nc.sbuf_tensor("colsum", [P, 1], F32))
        ones = ctx.enter_context(nc.sbuf_tensor("ones", [P, 1], F32))
        res = ctx.enter_context(nc.sbuf_tensor("res", [1, 1], F32))
        scratch = ctx.enter_context(nc.sbuf_tensor("scratch", [P, 1], F32))
        tot = ctx.enter_context(nc.psum_tensor("tot", [1, 1], F32))

        pk, lk, combo = ([] for _ in range(3))
        for k in range(chunks):
            pk.append(ctx.enter_context(
                nc.sbuf_tensor(f"pk{k}", [P, n], DT)))
            lk.append(ctx.enter_context(
                nc.sbuf_tensor(f"lk{k}", [P, n], DT)))
            # combo: [ek | nm] so one reduce covers both
            combo.append(ctx.enter_context(
                nc.sbuf_tensor(f"combo{k}", [P, 2 * n], DT)))

        def r3(ap, width):
            return ap.rearrange("p (c u) -> p c u", u=width)

        with nc.Block(no_gpsimd_drain=True) as block:

            @block.sync
            def _(sync):
                sync.dma_start(
                    out=idxt[:, 0:2 * cc], in_=idx[0:P, :],
                ).then_inc(sem_idx[0], 16)
                for k in range(2):
                    sync.dma_start(
                        out=lk[k][:, :],
                        in_=losses[k * P:(k + 1) * P, :],
                    ).then_inc(sem_l[k], 16)
                sync.wait_ge(sem_res2, 1)
                sync.dma_start(out=out[:, :], in_=res[:, :]).then_inc(
                    sem_out, 16)
                sync.wait_ge(sem_out, 16)

            @block.gpsimd
            def _(gpsimd):
                # warm up the SWDGE path (Q7 launch + queue start) with a
                # bounds-checked gather over an UNINITIALIZED index tensor:
                # garbage indices > V-1 are silently skipped, in-range ones
                # read harmlessly. No memset/wait needed -> issues at t0.
                gpsimd.indirect_dma_start(
                    out=wbuf[:, :],
                    out_offset=None,
                    in_=ptab[:, :],
                    in_offset=bass.IndirectOffsetOnAxis(
                        ap=widx[:, :], axis=0),
                    bounds_check=V - 1,
                    oob_is_err=False,
                ).then_inc(sem_w, 16)
                for k in range(chunks):
                    gpsimd.wait_ge(sem_idx[k // 2], 16)
                    gpsimd.indirect_dma_start(
                        out=pk[k][:, :],
                        out_offset=None,
                        in_=ptab[:, :],
                        in_offset=bass.IndirectOffsetOnAxis(
                            ap=idxt[:, k * cc:(k + 1) * cc], axis=0
                        ),
                    ).then_inc(sem_g[k], 16)

            @block.scalar
            def _(scalar):
                # second idx half + two loss chunks stream from the scalar
                # engine's queue set, in parallel with sync's.
                scalar.dma_start(
                    out=idxt[:, 2 * cc:4 * cc], in_=idx[P:2 * P, :],
                ).then_inc(sem_idx[1], 16)
                # dummy activation: forces the EXP table load off the
                # critical path (overlaps the idx DMA + gather).
                scalar.activation(
                    out=scratch[:, :], in_=scratch[:, :],
                    func=mybir.ActivationFunctionType.Exp,
                )
                for k in range(2, chunks):
                    scalar.dma_start(
                        out=lk[k][:, :],
                        in_=losses[k * P:(k + 1) * P, :],
                    ).then_inc(sem_l[k], 16)
                for k in range(chunks):
                    scalar.wait_ge(sem_g[k], 16)
                    scalar.activation(
                        out=combo[k][:, 0:n], in_=pk[k][:, :],
                        func=mybir.ActivationFunctionType.Exp,
                    ).then_inc(sem_exp, 1)

            # The DVE pipeline does not interlock same-engine RAW hazards:
            # every dependent pair needs an explicit wait on the engine's
            # completion counter.
            @block.vector
            def _(vector):
                state = {"n": 0, "hw": 0}

                def bump(inst):
                    state["n"] += 1
                    inst.then_inc(sem_dve, 1)
                    return state["n"]

                def dep(*ths):
                    th = max(ths)
                    if th > state["hw"]:
                        vector.wait_ge(sem_dve, th)
                        state["hw"] = th

                bump(vector.memset(ones[:, :], 1.0))
                i_red = [0] * chunks
                for k in range(chunks):
                    vector.wait_ge(sem_exp, k + 1)
                    vector.wait_ge(sem_l[k], 16)
                    i_mult = bump(vector.tensor_tensor(
                        out=combo[k][:, n:2 * n],
                        in0=combo[k][:, 0:n], in1=lk[k][:, :],
                        op=mybir.AluOpType.mult,
                    ))
                    dep(i_mult)
                    if USE_POOL:
                        i_red[k] = bump(vector.pool_avg(
                            out=dennum[:, k * 2 * cc:(k + 1) * 2 * cc],
                            in_=r3(combo[k][:, :], MAXC)[:, :, :],
                        ))
                    else:
                        i_red[k] = bump(vector.tensor_reduce(
                            out=dennum[:, k * 2 * cc:(k + 1) * 2 * cc],
                            in_=r3(combo[k][:, :], MAXC)[:, :, :],
                            axis=mybir.AxisListType.X,
                            op=mybir.AluOpType.add,
                        ))
                den_v = r3(dennum[:, :], 2 * cc)[:, :, 0:cc]
                num_v = r3(dennum[:, :], 2 * cc)[:, :, cc:2 * cc]
                rd_v = r3(rd[:, :], cc)
                acc_v = r3(acc[:, :], cc)
                dep(*i_red)
                if USE_RECIP_APPROX:
                    i_rd = bump(vector.reciprocal_approx_fast(
                        out=rd_v[:, :, :], in_=den_v))
                else:
                    i_rd = bump(vector.reciprocal(out=rd_v[:, :, :],
                                                  in_=den_v))
                dep(i_rd)
                i_acc = bump(vector.tensor_tensor(
                    out=acc_v[:, :, :], in0=num_v, in1=rd_v[:, :, :],
                    op=mybir.AluOpType.mult,
                ))
                dep(i_acc)
                vector.tensor_reduce(
                    out=colsum[:, :], in_=acc[:, :],
                    axis=mybir.AxisListType.X, op=mybir.AluOpType.add,
                ).then_inc(sem_res, 1)
                vector.wait_ge(sem_mm, 1)
                vector.tensor_copy(out=res[:, :], in_=tot[:, :]).then_inc(
                    sem_res2, 1)

            @block.tensor
            def _(tensor):
                tensor.wait_ge(sem_res, 1)
                tensor.matmul(
                    out=tot[:, :], lhsT=colsum[:, :], rhs=ones[:, :],
                    start=True, stop=True,
                ).then_inc(sem_mm, 1)

    return nc


def make_inputs(losses, inputs_idx, params, cardinality,
                chunks: int = CHUNKS, use_bf16: bool = USE_BF16,
                sort: bool = SORT):
    """Pack/shard full inputs into per-core input maps."""
    npdt = ml_dtypes.bfloat16 if use_bf16 else np.float32
    cc = COLS // chunks
    n = cc * MAXC
    p = np.asarray(params, dtype=np.float32)
    card = np.asarray(cardinality, dtype=np.int32)
    mask = np.arange(MAXC, dtype=np.int32)[None, :] < card[:, None]
    ptab = np.where(mask, p, np.float32(MASK_FILL)).astype(npdt)
    idx_full = np.asarray(inputs_idx, dtype=np.int32)
    losses_full = np.asarray(losses, dtype=np.float32)
    in_maps = []
    for c in range(NCORES):
        sl = slice(c * BC, (c + 1) * BC)
        idx_c = idx_full[sl]
        losses_c = losses_full[sl]
        if sort:
            order = np.argsort(idx_c)
            idx_c = idx_c[order]
            losses_c = losses_c[order]
        # chunk-major packing: chunk k holds sorted ranks
        # [k*P*cc, (k+1)*P*cc), laid out [P, cc] row-major; on SBUF the
        # chunk sits at idxt[:, k*cc:(k+1)*cc].
        idx_r = idx_c.reshape(chunks, P, cc)
        idx_packed = np.concatenate(
            [np.concatenate([idx_r[2 * h], idx_r[2 * h + 1]], axis=1)
             for h in range(chunks // 2)], axis=0)
        losses_packed = losses_c.reshape(chunks * P, n).astype(npdt)
        in_maps.append({
            "ptab": ptab,
            "idx": np.ascontiguousarray(idx_packed),
            "losses": np.ascontiguousarray(losses_packed),
        })
    return in_maps


_NC_CACHE = {}


def kernel(losses, inputs_idx, params, cardinality, trace=False, **kw):
    key = (CHUNKS, USE_BF16)
    if key not in _NC_CACHE:
        _NC_CACHE[key] = build_kernel(CHUNKS, USE_BF16)
    nc = _NC_CACHE[key]
    in_maps = make_inputs(losses, inputs_idx, params, cardinality,
                          CHUNKS, USE_BF16, SORT)
    r = run_bass_kernel_spmd(nc, in_maps, list(range(NCORES)), trace=trace, **kw)
    total = np.float64(0.0)
    for c in range(NCORES):
        total += np.float64(np.asarray(r.results[c]["out"],
                                       dtype=np.float32).sum(dtype=np.float64))
    out = np.float32(total)
    if trace:
        kernel.last_results = r
    return np.asarray(out)


kernel.last_results = None
